# revision 9
# baseline (speedup 1.0000x reference)
"""Trainium2 Bass kernel for the GeneGroupModel two-layer problem (v2).

Model: g = relu(segment_sum(x * w_flat, seg) + gene_b)
       h1 = relu(BN(g @ W1.T + b1));  h2 = relu(BN(h1 @ W2.T + b2))
       out = h2 @ Wout.T + bout            (BN uses full-batch statistics)

Strategy (8 NeuronCores, data-parallel over the batch):
 - batch B=2048 sharded 8 x 256 rows.
 - w_flat is folded into x on the host: xw = x * w_flat (bf16), and the
   segment-sum becomes IND.T @ xw_chunk with a single constant 0/1
   indicator IND [128, 15*64] reused by every super-chunk (the segment
   structure repeats every 1920 features == 64 groups).
 - x is pre-transposed and packed on the host to [128, NSUB*256] so the
   kernel streams it with 16 large (~1.9 MB) fully-contiguous DMAs
   (~15 KB per partition line) instead of 469 strided transpose-DMAs.
 - MLP layer 1 is accumulated incrementally: as each 128-group tile of
   gT completes, its 4 W1 matmuls are folded into persistent PSUM
   accumulators, so almost no MLP1 work remains after the x stream.
 - All matmuls run in bf16 (f32 PSUM accumulate). BN statistics are
   f32 and are summed across cores with two tiny AllReduces.
 - b1/b2 are omitted: BN subtracts the batch mean, so a constant bias
   added before BN cancels exactly.
"""

import numpy as np
import ml_dtypes

import concourse.bass as bass
import concourse.bacc as bacc
import concourse.mybir as mybir
from concourse import tile
from concourse.bass_utils import run_bass_kernel_spmd

F32 = mybir.dt.float32
BF16 = mybir.dt.bfloat16

B, F, G = 2048, 60000, 2000
H1, H2 = 512, 256
EPS = 1e-5
NCORES = 8
BS = B // NCORES            # 256 batch rows per core
NSUB = 469                  # ceil(F/128); F padded to FP
FP = NSUB * 128             # 60032
SUPER_SUBS = 15             # 15 x 128 = 1920 features per super-chunk
NSUPER = 32                 # 31 full + 1 tail (4 subchunks, 16 groups)
GBLK = 64                   # groups per full super-chunk
GT_TILES = 16               # partition tiles of gT (G padded to 2048)
NBLK = 16                   # DMA blocks (2 super-chunks each)
BLK_COLS = 2 * SUPER_SUBS * BS          # 7680 columns per full block
TAIL_COLS = (SUPER_SUBS + 4) * BS       # 4864 columns in block 15

_SIZES = np.tile(np.array([16, 24, 32, 48], np.int64), 500)


def _build_graph():
    nc = bacc.Bacc("TRN2", target_bir_lowering=False, debug=False,
                   num_devices=NCORES)
    x_d = nc.declare_dram_parameter("x", [128, NSUB * BS], BF16, isOutput=False)
    ind_d = nc.declare_dram_parameter("ind", [128, SUPER_SUBS * GBLK], BF16, isOutput=False)
    gbpt_d = nc.declare_dram_parameter("gbpt", [128, GT_TILES], F32, isOutput=False)
    w1t_d = nc.declare_dram_parameter("w1t", [128, GT_TILES * H1], BF16, isOutput=False)
    g1pt_d = nc.declare_dram_parameter("g1pt", [128, 4], F32, isOutput=False)
    be1pt_d = nc.declare_dram_parameter("be1pt", [128, 4], F32, isOutput=False)
    w2t_d = nc.declare_dram_parameter("w2t", [128, 4 * H2], BF16, isOutput=False)
    g2pt_d = nc.declare_dram_parameter("g2pt", [128, 2], F32, isOutput=False)
    be2pt_d = nc.declare_dram_parameter("be2pt", [128, 2], F32, isOutput=False)
    wopt_d = nc.declare_dram_parameter("wopt", [128, 2], BF16, isOutput=False)
    bout_d = nc.declare_dram_parameter("boutv", [1, 1], F32, isOutput=False)
    out_d = nc.declare_dram_parameter("out", [1, BS], F32, isOutput=True)

    AT = mybir.AluOpType
    AF = mybir.ActivationFunctionType
    AX = mybir.AxisListType

    with tile.TileContext(nc) as tc:
        with (
            tc.tile_pool(name="const", bufs=1) as constp,
            tc.tile_pool(name="xt", bufs=3) as xtp,
            tc.tile_pool(name="gt", bufs=1) as gtp,
            tc.tile_pool(name="mlp", bufs=1) as mlpp,
            tc.tile_pool(name="scratch", bufs=2) as scrp,
            tc.tile_pool(name="small", bufs=1) as smallp,
            tc.tile_pool(name="psg", bufs=2, space="PSUM") as psgp,
            tc.tile_pool(name="ph1", bufs=1, space="PSUM") as ph1p,
            tc.tile_pool(name="ph2", bufs=1, space="PSUM") as ph2p,
            tc.tile_pool(name="pso", bufs=1, space="PSUM") as psop,
            tc.tile_pool(name="dram", bufs=1, space="DRAM") as dramp,
        ):
            # ---------------- constants ----------------
            ind_sb = constp.tile([128, SUPER_SUBS * GBLK], BF16)
            nc.gpsimd.dma_start(ind_sb[:], ind_d[:])
            gbpt = constp.tile([128, GT_TILES], F32)
            nc.gpsimd.dma_start(gbpt[:], gbpt_d[:])
            w1t = constp.tile([128, GT_TILES * H1], BF16)
            nc.gpsimd.dma_start(w1t[:], w1t_d[:])
            w2t = constp.tile([128, 4 * H2], BF16)
            nc.gpsimd.dma_start(w2t[:], w2t_d[:])
            g1pt = constp.tile([128, 4], F32)
            nc.gpsimd.dma_start(g1pt[:], g1pt_d[:])
            be1pt = constp.tile([128, 4], F32)
            nc.gpsimd.dma_start(be1pt[:], be1pt_d[:])
            g2pt = constp.tile([128, 2], F32)
            nc.gpsimd.dma_start(g2pt[:], g2pt_d[:])
            be2pt = constp.tile([128, 2], F32)
            nc.gpsimd.dma_start(be2pt[:], be2pt_d[:])
            wopt = constp.tile([128, 2], BF16)
            nc.gpsimd.dma_start(wopt[:], wopt_d[:])
            boutv = constp.tile([1, 1], F32)
            nc.gpsimd.dma_start(boutv[:], bout_d[:])
            epst = constp.tile([128, 1], F32)
            nc.vector.memset(epst[:], EPS)

            # gT accumulator [2048(G padded) x 256] bf16: 16 partition
            # tiles side by side. Groups 2016..2047 (tail padding) are
            # never written -> zero the whole upper half of the last
            # tile (partition starts must be 32-aligned; rows 64..80
            # are overwritten by the tail super-chunk).
            gt = gtp.tile([128, GT_TILES * BS], BF16)
            nc.vector.memset(gt[64:128, 15 * BS:16 * BS], 0.0)

            # ---------------- segment-sum stream ----------------
            # Full blocks hold two super-chunks interleaved sub-chunk-wise
            # (host packing), so one [128, 64]-stationary load feeds a
            # 512-column matmul covering both super-chunks at once.
            for blk in range(NBLK):
                full = blk < NBLK - 1
                ncols = BLK_COLS if full else TAIL_COLS
                xblk = xtp.tile([128, BLK_COLS], BF16, tag="xblk")
                nc.sync.dma_start(xblk[:, 0:ncols],
                                  x_d[:, blk * BLK_COLS:blk * BLK_COLS + ncols])
                psg = psgp.tile([GBLK, 2 * BS], F32, tag="psg")
                if full:
                    for s in range(SUPER_SUBS):
                        nc.tensor.matmul(psg[:],
                                         ind_sb[:, s * GBLK:(s + 1) * GBLK],
                                         xblk[:, s * 2 * BS:(s + 1) * 2 * BS],
                                         start=(s == 0), stop=(s == SUPER_SUBS - 1))
                    ng2 = GBLK
                else:
                    # tail block: super-chunk 30 (15 subs), then 31 (4 subs)
                    for s in range(SUPER_SUBS):
                        nc.tensor.matmul(psg[:, 0:BS],
                                         ind_sb[:, s * GBLK:(s + 1) * GBLK],
                                         xblk[:, s * BS:(s + 1) * BS],
                                         start=(s == 0), stop=(s == SUPER_SUBS - 1))
                    for s in range(4):
                        nc.tensor.matmul(psg[:, BS:2 * BS],
                                         ind_sb[:, s * GBLK:(s + 1) * GBLK],
                                         xblk[:, (SUPER_SUBS + s) * BS:(SUPER_SUBS + s + 1) * BS],
                                         start=(s == 0), stop=(s == 3))
                    ng2 = 16
                # gt[...] = relu(psg + gene_b), cast to bf16
                nc.scalar.activation(gt[0:64, blk * BS:(blk + 1) * BS],
                                     psg[0:64, 0:BS], AF.Relu,
                                     bias=gbpt[0:64, blk:blk + 1])
                nc.scalar.activation(gt[64:64 + ng2, blk * BS:(blk + 1) * BS],
                                     psg[0:ng2, BS:2 * BS], AF.Relu,
                                     bias=gbpt[64:64 + ng2, blk:blk + 1])
            # ---------------- MLP layer 1 ----------------
            # sequential accumulation groups (one per m-tile); interleaving
            # long-lived PSUM groups with other matmuls corrupts them.
            h1ps = ph1p.tile([128, 4 * BS], F32)
            for m in range(4):
                for k in range(GT_TILES):
                    nc.tensor.matmul(
                        h1ps[:, m * BS:(m + 1) * BS],
                        w1t[:, k * H1 + m * 128:k * H1 + (m + 1) * 128],
                        gt[:, k * BS:(k + 1) * BS],
                        start=(k == 0), stop=(k == GT_TILES - 1))

            # ---------------- BN1 stats + AllReduce ----------------
            stats1 = smallp.tile([128, 8], F32)
            for m in range(4):
                nc.vector.reduce_sum(stats1[:, m:m + 1],
                                     h1ps[:, m * BS:(m + 1) * BS], axis=AX.X)
                sq = scrp.tile([128, BS], F32, tag="sq")
                nc.scalar.activation(sq[:], h1ps[:, m * BS:(m + 1) * BS],
                                     AF.Square,
                                     accum_out=stats1[:, 4 + m:5 + m])

            bn1_in = dramp.tile([128, 8], F32)
            bn1_out = dramp.tile([128, 8], F32)
            nc.sync.dma_start(bn1_in[:], stats1[:])
            nc.gpsimd.collective_compute(
                "AllReduce", AT.add,
                replica_groups=[list(range(NCORES))],
                ins=[bn1_in.opt()], outs=[bn1_out.opt()])
            statsr1 = smallp.tile([128, 8], F32)
            nc.sync.dma_start(statsr1[:], bn1_out[:])

            mu1 = smallp.tile([128, 4], F32)
            nc.vector.tensor_scalar_mul(mu1[:], statsr1[:, 0:4], 1.0 / B)
            var1 = smallp.tile([128, 4], F32)
            nc.vector.tensor_tensor(var1[:], mu1[:], mu1[:], op=AT.mult)
            ex21 = smallp.tile([128, 4], F32)
            nc.vector.tensor_scalar_mul(ex21[:], statsr1[:, 4:8], 1.0 / B)
            nc.vector.tensor_tensor(var1[:], ex21[:], var1[:], op=AT.subtract)
            std1 = smallp.tile([128, 4], F32)
            nc.scalar.activation(std1[:], var1[:], AF.Sqrt, bias=epst[:])
            rstd1 = smallp.tile([128, 4], F32)
            nc.vector.reciprocal(rstd1[:], std1[:])
            scl1 = smallp.tile([128, 4], F32)
            nc.vector.tensor_tensor(scl1[:], g1pt[:], rstd1[:], op=AT.mult)
            shf1 = smallp.tile([128, 4], F32)
            nc.vector.tensor_tensor(shf1[:], mu1[:], scl1[:], op=AT.mult)
            nc.vector.tensor_tensor(shf1[:], be1pt[:], shf1[:], op=AT.subtract)

            h1 = mlpp.tile([128, 4 * BS], BF16)
            for m in range(4):
                nc.scalar.activation(
                    h1[:, m * BS:(m + 1) * BS], h1ps[:, m * BS:(m + 1) * BS],
                    AF.Relu, bias=shf1[:, m:m + 1], scale=scl1[:, m:m + 1])

            # ---------------- MLP layer 2 + BN2 ----------------
            h2ps = ph2p.tile([128, 2 * BS], F32)
            for m in range(2):
                for k in range(4):
                    nc.tensor.matmul(
                        h2ps[:, m * BS:(m + 1) * BS],
                        w2t[:, k * H2 + m * 128:k * H2 + (m + 1) * 128],
                        h1[:, k * BS:(k + 1) * BS],
                        start=(k == 0), stop=(k == 3))
            stats2 = smallp.tile([128, 4], F32)
            for m in range(2):
                nc.vector.reduce_sum(stats2[:, m:m + 1],
                                     h2ps[:, m * BS:(m + 1) * BS], axis=AX.X)
                sq2 = scrp.tile([128, BS], F32, tag="sq")
                nc.scalar.activation(sq2[:], h2ps[:, m * BS:(m + 1) * BS],
                                     AF.Square,
                                     accum_out=stats2[:, 2 + m:3 + m])

            bn2_in = dramp.tile([128, 4], F32)
            bn2_out = dramp.tile([128, 4], F32)
            nc.sync.dma_start(bn2_in[:], stats2[:])
            nc.gpsimd.collective_compute(
                "AllReduce", AT.add,
                replica_groups=[list(range(NCORES))],
                ins=[bn2_in.opt()], outs=[bn2_out.opt()])
            statsr2 = smallp.tile([128, 4], F32)
            nc.sync.dma_start(statsr2[:], bn2_out[:])

            mu2 = smallp.tile([128, 2], F32)
            nc.vector.tensor_scalar_mul(mu2[:], statsr2[:, 0:2], 1.0 / B)
            var2 = smallp.tile([128, 2], F32)
            nc.vector.tensor_tensor(var2[:], mu2[:], mu2[:], op=AT.mult)
            ex22 = smallp.tile([128, 2], F32)
            nc.vector.tensor_scalar_mul(ex22[:], statsr2[:, 2:4], 1.0 / B)
            nc.vector.tensor_tensor(var2[:], ex22[:], var2[:], op=AT.subtract)
            std2 = smallp.tile([128, 2], F32)
            nc.scalar.activation(std2[:], var2[:], AF.Sqrt, bias=epst[:])
            rstd2 = smallp.tile([128, 2], F32)
            nc.vector.reciprocal(rstd2[:], std2[:])
            scl2 = smallp.tile([128, 2], F32)
            nc.vector.tensor_tensor(scl2[:], g2pt[:], rstd2[:], op=AT.mult)
            shf2 = smallp.tile([128, 2], F32)
            nc.vector.tensor_tensor(shf2[:], mu2[:], scl2[:], op=AT.mult)
            nc.vector.tensor_tensor(shf2[:], be2pt[:], shf2[:], op=AT.subtract)

            h2 = mlpp.tile([128, 2 * BS], BF16)
            for m in range(2):
                nc.scalar.activation(
                    h2[:, m * BS:(m + 1) * BS], h2ps[:, m * BS:(m + 1) * BS],
                    AF.Relu, bias=shf2[:, m:m + 1], scale=scl2[:, m:m + 1])

            # ---------------- output head ----------------
            pso = psop.tile([1, BS], F32)
            for k in range(2):
                nc.tensor.matmul(pso[:], wopt[:, k:k + 1],
                                 h2[:, k * BS:(k + 1) * BS],
                                 start=(k == 0), stop=(k == 1))
            outsb = smallp.tile([1, BS], F32)
            nc.scalar.activation(outsb[:], pso[:], AF.Identity,
                                 bias=boutv[0:1, 0:1])
            nc.sync.dma_start(out_d[:], outsb[:])

    nc.compile()
    return nc


def _pack_pt(v, ncols):
    """[N] -> [128, ncols] with element (p, c) = v[128c + p], zero padded."""
    full = np.zeros(128 * ncols, np.float32)
    full[:v.shape[0]] = v
    return np.ascontiguousarray(full.reshape(ncols, 128).T)


def _pack_rows(mat, ntiles):
    """[ntiles*128, C] -> [128, ntiles*C] with (p, k*C + c) = mat[128k + p, c]."""
    C = mat.shape[1]
    return np.ascontiguousarray(
        mat.reshape(ntiles, 128, C).transpose(1, 0, 2)).reshape(128, ntiles * C)


_GRAPH = None


def _prepare_in_maps(x, seg, w_flat, gene_b, W1, b1, gamma1, beta1, W2, b2,
                     gamma2, beta2, Wout, bout):
    x = np.asarray(x, np.float32)
    seg = np.asarray(seg)
    exp_seg = np.repeat(np.arange(G, dtype=np.int64), _SIZES)
    assert np.array_equal(seg.astype(np.int64), exp_seg), "unexpected seg layout"

    # fold w_flat into x, cast bf16, pad features, transpose+pack per core
    xw = (x * np.asarray(w_flat, np.float32)[None, :]).astype(ml_dtypes.bfloat16)
    xpad = np.zeros((B, FP), np.uint16)
    xpad[:, :F] = xw.view(np.uint16)
    # sub-chunk order: full blocks interleave their two super-chunks
    # sub-chunk-wise (s0|s0', s1|s1', ...); tail block stays sequential
    order = []
    for b in range(NBLK - 1):
        for s in range(SUPER_SUBS):
            order += [30 * b + s, 30 * b + SUPER_SUBS + s]
    order += list(range(30 * (NBLK - 1), NSUB))
    order = np.asarray(order)

    xps = []
    for c in range(NCORES):
        A2 = np.ascontiguousarray(xpad[c * BS:(c + 1) * BS].T)   # [FP, BS]
        xp = _pack_rows(A2, NSUB)                                # [128, NSUB*BS]
        xp = np.ascontiguousarray(
            xp.reshape(128, NSUB, BS)[:, order, :]).reshape(128, NSUB * BS)
        xps.append(xp.view(ml_dtypes.bfloat16))

    ind = (exp_seg[:SUPER_SUBS * 128].reshape(SUPER_SUBS, 128)[:, :, None]
           == np.arange(GBLK)[None, None, :])                    # [15, 128, 64]
    ind_pack = np.ascontiguousarray(
        ind.transpose(1, 0, 2)).reshape(128, SUPER_SUBS * GBLK)
    ind_pack = ind_pack.astype(ml_dtypes.bfloat16)

    gbpt = _pack_pt(np.asarray(gene_b, np.float32), GT_TILES)
    w1f = np.zeros((GT_TILES * 128, H1), np.float32)
    w1f[:G] = np.asarray(W1, np.float32).T
    w1t = _pack_rows(w1f, GT_TILES).astype(ml_dtypes.bfloat16)
    w2t = _pack_rows(np.ascontiguousarray(np.asarray(W2, np.float32).T),
                     4).astype(ml_dtypes.bfloat16)
    g1pt = _pack_pt(np.asarray(gamma1, np.float32), 4)
    be1pt = _pack_pt(np.asarray(beta1, np.float32), 4)
    g2pt = _pack_pt(np.asarray(gamma2, np.float32), 2)
    be2pt = _pack_pt(np.asarray(beta2, np.float32), 2)
    wopt = _pack_pt(np.asarray(Wout, np.float32).reshape(-1),
                    2).astype(ml_dtypes.bfloat16)
    boutv = np.asarray(bout, np.float32).reshape(1, 1)

    consts = dict(ind=ind_pack, gbpt=gbpt, w1t=w1t, g1pt=g1pt,
                  be1pt=be1pt, w2t=w2t, g2pt=g2pt, be2pt=be2pt,
                  wopt=wopt, boutv=boutv)
    return [dict(consts, x=xps[i]) for i in range(NCORES)]


def _graph():
    global _GRAPH
    if _GRAPH is None:
        _GRAPH = _build_graph()
    return _GRAPH


def _gather(res):
    out = np.concatenate([np.asarray(r["out"]).reshape(-1)
                          for r in res.results])
    return out.reshape(B, 1).astype(np.float32)


def kernel(**inputs):
    in_maps = _prepare_in_maps(**inputs)
    res = run_bass_kernel_spmd(_graph(), in_maps, list(range(NCORES)))
    return _gather(res)
